# revision 3
# baseline (speedup 1.0000x reference)
"""Bass/Trainium2 kernel for a single-head causal decoder attention head.

Reference computation (fp32):
    k = x @ Wk; q = x @ Wq; v = x @ Wv            # [B,T,H]
    att = softmax(causal(q k^T / sqrt(H)))        # [B,T,T]
    out = att @ v                                 # [B,T,H]
with B=4, T=4096, C=1024, H=128.

Sharding: 8 cores = 4 batches x 2 query-interleave lanes (j in {0,1}).
Core (b, j) handles q-blocks {(2i+j)*512 : i in 0..3}.  The host hands
each core a *permuted* x^T whose columns are [own-lane blocks | other-
lane blocks], so every core runs one identical instruction stream
(SPMD): Q is projected from the first four 512-col groups only, and
attention group i scans a uniform kv span of 4(i+1) chunks in the own
section plus 4(i+1) chunks in the other section.  Causality reduces to
a lane-independent triangular mask on the diagonal block plus a
per-lane all-0/all-1 mask on the final 4 other-section chunks.

Dataflow (per core, transposed land - no on-chip transposes at all):
    KT [H, T]    = Wk^T xp^T          (8 matmuls of N=512 per 512 cols)
    QT [H, 2048] = Wq^T xp^T          (own section only)
    VV [kv, H]   (32 chunk blocks)    (lhsT = xp^T 128x128 block, rhs = Wv chunk)
    per q-group i, per kv pair-of-chunks (pt tile [128kv x 1024]):
        S^T  = KT_c^T QT_i            (PSUM [128, 2*512])
        P^T  = exp(S^T / sqrt(H))     (ACT, fp32 PSUM -> bf16 SBUF)
        P^T *= mask                   (DVE, last 2 tiles of each section)
        outT += VV_c^T P^T            (PSUM [128H, 512q], accumulated)
        fold  = P^T[:, :512] + P^T[:, 512:]   (DVE)
        sums += ones^T fold           (PSUM [128, 512q], accumulated)
    outT / sums -> DRAM  (reciprocal + multiply)

Projections for kv groups 5..7 are interleaved between attention groups
0..2 so DMA/ACT/DVE overlap the tensor engine throughout.
"""

import sys

sys.path.insert(0, "/opt/trn_rl_repo")

import numpy as np
import ml_dtypes

import concourse.mybir as mybir
import concourse.tile as tile
from concourse import bacc
from concourse.alu_op_type import AluOpType
from concourse.bass_utils import run_bass_kernel_spmd

B, T, C, H = 4, 4096, 1024, 128
NCORES = 8
QG = 512                      # q-group width
NG = 4                        # q-groups per core
CB = C // 128                 # 8 contraction chunks
TGRP = T // QG                # 8 column groups of x^T
SCALE = float(H) ** -0.5

BF16 = mybir.dt.bfloat16
F32 = mybir.dt.float32
NPBF16 = ml_dtypes.bfloat16


def _build_program():
    nc = bacc.Bacc("TRN2", target_bir_lowering=False, debug=False)

    xt = nc.dram_tensor("xt", [C, T], BF16, kind="ExternalInput").ap()
    wk = nc.dram_tensor("wk", [C, H], BF16, kind="ExternalInput").ap()
    wq = nc.dram_tensor("wq", [C, H], BF16, kind="ExternalInput").ap()
    wv = nc.dram_tensor("wv", [C, H], BF16, kind="ExternalInput").ap()
    msk = nc.dram_tensor("msk", [128, 8 * QG], BF16, kind="ExternalInput").ap()
    outT = nc.dram_tensor("outT", [H, NG * QG], F32, kind="ExternalOutput").ap()

    with tile.TileContext(nc) as tc:
        with (
            tc.tile_pool(name="const", bufs=1) as constp,
            tc.tile_pool(name="kvq", bufs=1) as kvqp,
            tc.tile_pool(name="xin", bufs=3) as xinp,
            tc.tile_pool(name="attb", bufs=4) as attp,
            tc.tile_pool(name="foldb", bufs=4) as foldp,
            tc.tile_pool(name="epi", bufs=2) as epip,
            tc.tile_pool(name="pp", bufs=2, space="PSUM") as ppool,
            tc.tile_pool(name="ap", bufs=1, space="PSUM") as apool,
        ):
            # --- persistent SBUF tensors ---
            wks = constp.tile([128, CB * H], BF16, tag="wks")
            wqs = constp.tile([128, CB * H], BF16, tag="wqs")
            wvs = constp.tile([128, CB * H], BF16, tag="wvs")
            for eng, ws, w in (
                (nc.scalar, wks, wk), (nc.scalar, wqs, wq), (nc.gpsimd, wvs, wv)
            ):
                eng.dma_start(
                    ws.rearrange("p (c h) -> p c h", c=CB),
                    w.rearrange("(c p) h -> p c h", p=128),
                )
            masks = constp.tile([128, 8 * QG], BF16, tag="masks")
            nc.scalar.dma_start(masks, msk)

            KT = kvqp.tile([128, T], BF16, tag="KT")
            VV = kvqp.tile([128, T], BF16, tag="VV")
            QT = kvqp.tile([128, NG * QG], BF16, tag="QT")
            ones = kvqp.tile([128, 128], BF16, tag="ones")
            nc.vector.memset(ones, 1.0)

            xtr = xt.rearrange("(c p) t -> p c t", p=128)

            def proj(tg, with_q):
                xg = xinp.tile([128, CB * QG], BF16, tag="xg", bufs=3)
                xgv = xg.rearrange("p (c q) -> p c q", c=CB)
                nc.sync.dma_start(xgv, xtr[:, :, tg * QG:(tg + 1) * QG])
                kps = ppool.tile([128, QG], F32, tag="pps")
                for ci in range(CB):
                    nc.tensor.matmul(
                        kps,
                        lhsT=wks[:, ci * H:(ci + 1) * H],
                        rhs=xg[:, ci * QG:(ci + 1) * QG],
                        start=(ci == 0),
                        stop=(ci == CB - 1),
                    )
                nc.any.tensor_copy(KT[:, tg * QG:(tg + 1) * QG], kps)
                vps = ppool.tile([128, QG], F32, tag="pps")
                for tb in range(QG // 128):
                    for ci in range(CB):
                        nc.tensor.matmul(
                            vps[:, tb * 128:(tb + 1) * 128],
                            lhsT=xgv[:, ci, tb * 128:(tb + 1) * 128],
                            rhs=wvs[:, ci * H:(ci + 1) * H],
                            start=(ci == 0),
                            stop=(ci == CB - 1),
                        )
                nc.any.tensor_copy(VV[:, tg * QG:(tg + 1) * QG], vps)
                if with_q:
                    qps = ppool.tile([128, QG], F32, tag="pps")
                    for ci in range(CB):
                        nc.tensor.matmul(
                            qps,
                            lhsT=wqs[:, ci * H:(ci + 1) * H],
                            rhs=xg[:, ci * QG:(ci + 1) * QG],
                            start=(ci == 0),
                            stop=(ci == CB - 1),
                        )
                    nc.any.tensor_copy(QT[:, tg * QG:(tg + 1) * QG], qps)

            def att(i):
                qg = QT[:, i * QG:(i + 1) * QG]
                otps = apool.tile([128, QG], F32, tag="otps", bufs=1)
                smps = apool.tile([128, QG], F32, tag="smps", bufs=1)
                ntiles = 2 * (i + 1)
                for sec in range(2):
                    cbase = 16 * sec
                    for tp in range(ntiles):
                        c0 = cbase + 2 * tp
                        sps = apool.tile([128, 2 * QG], F32, tag="sps", bufs=2)
                        for h in range(2):
                            nc.tensor.matmul(
                                sps[:, h * QG:(h + 1) * QG],
                                lhsT=KT[:, (c0 + h) * 128:(c0 + h + 1) * 128],
                                rhs=qg,
                                start=True,
                                stop=True,
                            )
                        pt = attp.tile([128, 2 * QG], BF16, tag="pt")
                        nc.scalar.activation(
                            pt, sps, mybir.ActivationFunctionType.Exp, scale=SCALE
                        )
                        mt = tp - (ntiles - 2)
                        if mt >= 0:
                            moff = sec * 4 * QG + mt * 2 * QG
                            nc.vector.tensor_tensor(
                                pt, pt, masks[:, moff:moff + 2 * QG],
                                op=AluOpType.mult,
                            )
                        first = (sec == 0 and tp == 0)
                        last = (sec == 1 and tp == ntiles - 1)
                        for h in range(2):
                            c = c0 + h
                            nc.tensor.matmul(
                                otps,
                                lhsT=VV[:, c * 128:(c + 1) * 128],
                                rhs=pt[:, h * QG:(h + 1) * QG],
                                start=(first and h == 0),
                                stop=(last and h == 1),
                            )
                        fold = foldp.tile([128, QG], BF16, tag="fold")
                        nc.vector.tensor_tensor(
                            fold, pt[:, 0:QG], pt[:, QG:2 * QG], op=AluOpType.add
                        )
                        nc.tensor.matmul(smps, lhsT=ones, rhs=fold,
                                         start=first, stop=last)
                rb = epip.tile([128, QG], F32, tag="rb")
                nc.vector.reciprocal_approx_fast(rb, smps)
                ot = epip.tile([128, QG], F32, tag="ot")
                nc.vector.tensor_tensor(ot, otps, rb, op=AluOpType.mult)
                nc.sync.dma_start(outT[:, i * QG:(i + 1) * QG], ot)

            for tg in range(5):
                proj(tg, with_q=(tg < NG))
            att(0)
            for k in range(1, NG):
                proj(4 + k, with_q=False)
                att(k)

    if not nc.is_finalized():
        nc.finalize()
    return nc


_NC_CACHE = None


def _get_program():
    global _NC_CACHE
    if _NC_CACHE is None:
        _NC_CACHE = _build_program()
    return _NC_CACHE


def _make_masks(j: int) -> np.ndarray:
    """Multiplicative mask [128, 4096] for lane j.

    Cols [0, 2048): triangular masks for the 4 chunks of the own-section
    diagonal block (chunk c masked where 128*c + kv > q), lane-independent.
    Cols [2048, 4096): pad mask for the final 4 other-section chunks -
    all-zero for lane 0 (padded block), all-one for lane 1 (real block).
    """
    out = np.empty((128, 8 * QG), np.float32)
    kv = np.arange(128)[:, None]
    q = np.arange(QG)[None, :]
    for c in range(4):
        out[:, c * QG:(c + 1) * QG] = (128 * c + kv <= q)
    out[:, 4 * QG:] = float(j)
    return out.astype(NPBF16)


def _run(inputs: dict, trace: bool = False, trace_kwargs: dict | None = None):
    x = np.asarray(inputs["x"], np.float32)
    Wk = np.asarray(inputs["Wk"], np.float32)
    Wq = np.asarray(inputs["Wq"], np.float32)
    Wv = np.asarray(inputs["Wv"], np.float32)

    nc = _get_program()

    wk16 = Wk.astype(NPBF16)
    wq16 = Wq.astype(NPBF16)
    wv16 = Wv.astype(NPBF16)
    msks = [_make_masks(j) for j in range(2)]

    in_maps = []
    for b in range(B):
        xtb = np.ascontiguousarray(x[b].T).astype(NPBF16)  # [C, T]
        for j in range(2):
            xtp = np.concatenate(
                [xtb[:, (2 * i + j) * QG:(2 * i + j + 1) * QG] for i in range(NG)]
                + [xtb[:, (2 * i + 1 - j) * QG:(2 * i + 2 - j) * QG]
                   for i in range(NG)],
                axis=1,
            )
            in_maps.append(
                {
                    "xt": np.ascontiguousarray(xtp),
                    "wk": wk16,
                    "wq": wq16,
                    "wv": wv16,
                    "msk": msks[j],
                }
            )

    res = run_bass_kernel_spmd(
        nc,
        in_maps,
        core_ids=list(range(NCORES)),
        trace=trace,
        **(trace_kwargs or {}),
    )

    out = np.empty((B, T, H), np.float32)
    for core in range(NCORES):
        b, j = divmod(core, 2)
        oT = np.asarray(res.results[core]["outT"], np.float32)  # [H, NG*QG]
        for i in range(NG):
            g = (2 * i + j) * QG
            out[b, g:g + QG, :] = oT[:, i * QG:(i + 1) * QG].T
    return out, res


def kernel(**inputs) -> np.ndarray:
    out, _ = _run(inputs, trace=False)
    return out
